# revision 6
# baseline (speedup 1.0000x reference)
"""BERT-style MLM forward on 8 TRN2 NeuronCores.

Strategy: pure data-parallel over batch (B=8 -> 1 sequence per core, no
collectives). Feature-major activations [dm, t] so every GEMM uses the
natural weight layout as the stationary operand. bf16 matmul operands with
fp32 PSUM accumulation; LN/softmax statistics in fp32. Embedding lookup and
masked-position gather are done as one-hot matmuls (one-hots built on host).
The final loss scalar is derived on host from the device-computed logits
(identical math to the reference, ~10K FLOPs).
"""
import sys

try:
    import concourse.bass as bass  # noqa: F401
except ImportError:
    sys.path.insert(0, "/opt/trn_rl_repo")

import ml_dtypes
import numpy as np

import concourse.bass as bass
import concourse.mybir as mybir
from concourse import bacc, tile
from concourse.bass_utils import run_bass_kernel_spmd
from concourse.masks import make_identity

B, T, NM = 8, 256, 40
V, NE, DM, DK, DVh, H, DFF, L = 30, 1024, 1024, 64, 64, 16, 4096, 8
EPS = 1e-5
P = 128
CD = DM // P   # 8 chunks of the model dim
CF = DFF // P  # 32 chunks of the ffn dim
TQ = T // P    # 2 token tiles
SCALE = 1.0 / np.sqrt(DK).astype(np.float32)  # 0.125
MASKVAL = float(-1e9 / SCALE)

bf16 = mybir.dt.bfloat16
f32 = mybir.dt.float32
BF = ml_dtypes.bfloat16
AF = mybir.ActivationFunctionType
OP = mybir.AluOpType
AX = mybir.AxisListType


def build_graph(debug=False):
    nc = bacc.Bacc()
    dp = {}
    dbg_outs = {}

    def param(name, shape, dt=bf16):
        dp[name] = nc.declare_dram_parameter(name, list(shape), dt, isOutput=False)
        return dp[name]

    # per-core tensors
    seq_oh = param("seq_oh", (V, T))            # one-hot of masked_seqs[b]
    pos_oh = param("pos_oh", (T, NM))           # one-hot of masked_pos[b]
    mask4 = param("mask4", (P, 4, T), f32)      # additive pad mask / SCALE, replicated
    # weights (shared across cores)
    te_w = param("te_w", (V, NE))
    pos_eT = param("pos_eT", (NE, T))           # pos_embed[0].T (feature-major)
    fc_w = param("fc_w", (NE, DM))
    wq_w = param("wq_w", (L, DM, H * DK))
    wk_w = param("wk_w", (L, DM, H * DK))
    wv_w = param("wv_w", (L, DM, H * DVh))
    ao_w = param("ao_w", (L, H * DVh, DM))
    f1_w = param("f1_w", (L, DM, DFF))
    f2_w = param("f2_w", (L, DFF, DM))
    lin_w = param("lin_w", (DM, DM))
    head_w = param("head_w", (DM, V))
    # fp32 per-partition-layout vectors [P, nchunks]
    norm_g = param("norm_g", (P, CD), f32)
    norm_b = param("norm_b", (P, CD), f32)
    fc_b = param("fc_b", (P, CD), f32)
    ln1_g = param("ln1_g", (L, P, CD), f32)
    ln1_b = param("ln1_b", (L, P, CD), f32)
    ln2_g = param("ln2_g", (L, P, CD), f32)
    ln2_b = param("ln2_b", (L, P, CD), f32)
    wq_b = param("wq_b", (L, P, CD), f32)
    wk_b = param("wk_b", (L, P, CD), f32)
    wv_b = param("wv_b", (L, P, CD), f32)
    ao_b = param("ao_b", (L, P, CD), f32)
    f1_b = param("f1_b", (L, P, CF), f32)
    f2_b = param("f2_b", (L, P, CD), f32)
    lnf_g = param("lnf_g", (P, CD), f32)
    lnf_b = param("lnf_b", (P, CD), f32)
    lin_b = param("lin_b", (P, CD), f32)
    logits_out = nc.declare_dram_parameter("logitsT", [V, NM], f32, isOutput=True)

    def mkdump(name, shape):
        if debug:
            dbg_outs[name] = nc.declare_dram_parameter("dbg_" + name, list(shape), bf16, isOutput=True)

    for nm in ("emb", "e_n", "x0", "h0", "q0", "k0", "ctx0", "x2_0", "h2_0", "x1", "xf"):
        if debug:
            dbg_outs[nm] = nc.declare_dram_parameter("dbg_" + nm, [P, CD, T], bf16, isOutput=True)
    if debug:
        dbg_outs["vtok0"] = nc.declare_dram_parameter("dbg_vtok0", [P, TQ, H * DVh], bf16, isOutput=True)
        dbg_outs["xtok"] = nc.declare_dram_parameter("dbg_xtok", [P, TQ, DM], bf16, isOutput=True)
        dbg_outs["sel"] = nc.declare_dram_parameter("dbg_sel", [P, CD, NM], bf16, isOutput=True)
        dbg_outs["hg"] = nc.declare_dram_parameter("dbg_hg", [P, CD, NM], bf16, isOutput=True)
        dbg_outs["at0"] = nc.declare_dram_parameter("dbg_at0", [P, 4, T], bf16, isOutput=True)

    with tile.TileContext(nc) as tc:
        with tc.tile_pool(name="cst", bufs=1) as cst, \
             tc.tile_pool(name="acts", bufs=2) as acts, \
             tc.tile_pool(name="wsl", bufs=6) as wslp, \
             tc.tile_pool(name="sm", bufs=2) as sm, \
             tc.tile_pool(name="psA", bufs=1, space="PSUM") as psA, \
             tc.tile_pool(name="psB", bufs=1, space="PSUM") as psB:

            # ---- constants ----
            ident = cst.tile([P, P], bf16, tag="ident")
            make_identity(nc, ident)
            onesk = cst.tile([P, 1], bf16, tag="onesk")
            nc.vector.memset(onesk, 1.0)
            ones1 = cst.tile([1, P], bf16, tag="ones1")
            nc.vector.memset(ones1, 1.0)
            eps_t = cst.tile([1, 1], f32, tag="eps")
            nc.vector.memset(eps_t, EPS)
            mask_sb = cst.tile([P, 4, T], f32, tag="mask")
            nc.sync.dma_start(out=mask_sb, in_=mask4[:, :, :])

            def dump(name, tile_ap):
                if debug and name in dbg_outs:
                    nc.sync.dma_start(out=dbg_outs[name][:], in_=tile_ap)

            def ldvec(ap2d):
                t = sm.tile([P, ap2d.shape[-1]], f32, tag="vecs")
                nc.sync.dma_start(out=t, in_=ap2d)
                return t

            # ---- layernorm over the feature (partition-chunk) axis ----
            def layer_norm(x, g, b, out):
                sq = sm.tile([P, CD, T], bf16, tag="lnsq")
                nc.vector.tensor_tensor(out=sq[:, :, :], in0=x[:, :, :], in1=x[:, :, :], op=OP.mult)
                st = psB.tile([1, 2, T], f32, tag="tp")
                for c in range(CD):
                    nc.tensor.matmul(st[:, 0, :], lhsT=onesk[:, :], rhs=x[:, c, :],
                                     start=(c == 0), stop=(c == CD - 1))
                for c in range(CD):
                    nc.tensor.matmul(st[:, 1, :], lhsT=onesk[:, :], rhs=sq[:, c, :],
                                     start=False, stop=(c == CD - 1))
                m_f = sm.tile([1, T], f32, tag="ln_mf")
                nc.vector.tensor_scalar_mul(m_f[:, :], st[:, 0, :], 1.0 / DM)
                ms_f = sm.tile([1, T], f32, tag="ln_msf")
                nc.vector.tensor_scalar_mul(ms_f[:, :], st[:, 1, :], 1.0 / DM)
                m2 = sm.tile([1, T], f32, tag="ln_m2")
                nc.vector.tensor_tensor(out=m2[:, :], in0=m_f[:, :], in1=m_f[:, :], op=OP.mult)
                nc.vector.tensor_tensor(out=ms_f[:, :], in0=ms_f[:, :], in1=m2[:, :], op=OP.subtract)
                std = sm.tile([1, T], f32, tag="ln_std")
                nc.scalar.activation(out=std[:, :], in_=ms_f[:, :], func=AF.Sqrt,
                                     bias=eps_t[:1, :], scale=1.0)
                nc.vector.reciprocal(out=std[:, :], in_=std[:, :])
                rows = sm.tile([1, 2, T], bf16, tag="ln_rows")
                nc.vector.tensor_copy(rows[:, 0, :], m_f[:, :])
                nc.vector.tensor_copy(rows[:, 1, :], std[:, :])
                bc = psB.tile([P, 2, T], f32, tag="sc")
                nc.tensor.matmul(bc[:, 0, :], lhsT=ones1[:, :], rhs=rows[:, 0, :], start=True, stop=True)
                nc.tensor.matmul(bc[:, 1, :], lhsT=ones1[:, :], rhs=rows[:, 1, :], start=True, stop=True)
                xc = sm.tile([P, CD, T], bf16, tag="ln_xc")
                for c in range(CD):
                    nc.vector.tensor_tensor(out=xc[:, c, :], in0=x[:, c, :], in1=bc[:, 0, :], op=OP.subtract)
                    nc.vector.tensor_tensor(out=xc[:, c, :], in0=xc[:, c, :], in1=bc[:, 1, :], op=OP.mult)
                    nc.scalar.activation(out=out[:, c, :], in_=xc[:, c, :], func=AF.Identity,
                                         bias=b[:, c:c + 1], scale=g[:, c:c + 1])

            # ---- generic feature-major GEMM block: out chunks = W[cin*P, mout*P].T @ act ----
            def gemm(w_dram, act_in, cin, mout, evict):
                nps = (mout + 1) // 2
                ps = [psA.tile([P, 2, T], f32, tag=f"gps{i}", name=f"gps{i}") for i in range(nps)]
                for c in range(cin):
                    wsl = wslp.tile([P, mout * P], bf16, tag="wslab")
                    nc.sync.dma_start(out=wsl, in_=w_dram[c * P:(c + 1) * P, :])
                    for m in range(mout):
                        nc.tensor.matmul(ps[m // 2][:, m % 2, :], lhsT=wsl[:, m * P:(m + 1) * P],
                                         rhs=act_in[:, c, :], start=(c == 0 and m % 2 == 0),
                                         stop=(c == cin - 1 and (m % 2 == 1 or m == mout - 1)))
                for m in range(mout):
                    evict(m, ps[m // 2][:, m % 2, :])

            # ---- embedding ----
            seq_sb = cst.tile([V, T], bf16, tag="seq")
            nc.sync.dma_start(out=seq_sb, in_=seq_oh[:, :])
            te_sb = cst.tile([V, NE], bf16, tag="te")
            nc.sync.dma_start(out=te_sb, in_=te_w[:, :])
            pos_sb = cst.tile([P, CD, T], bf16, tag="pos")
            nc.sync.dma_start(out=pos_sb, in_=pos_eT[:, :].rearrange("(c p) t -> p c t", p=P))

            emb = acts.tile([P, CD, T], bf16, tag="emb")
            eps_ps = [psA.tile([P, 2, T], f32, tag=f"gps{i}", name=f"gps{i}") for i in range(4)]
            for c in range(CD):
                nc.tensor.matmul(eps_ps[c // 2][:, c % 2, :], lhsT=te_sb[:, c * P:(c + 1) * P],
                                 rhs=seq_sb[:, :], start=True, stop=True)
            for c in range(CD):
                nc.vector.tensor_tensor(out=emb[:, c, :], in0=eps_ps[c // 2][:, c % 2, :],
                                        in1=pos_sb[:, c, :], op=OP.add)

            dump("emb", emb[:, :, :])
            ng = ldvec(norm_g[:, :]); nb = ldvec(norm_b[:, :])
            e_n = acts.tile([P, CD, T], bf16, tag="e_n")
            layer_norm(emb, ng, nb, e_n)

            dump("e_n", e_n[:, :, :])
            fcb = ldvec(fc_b[:, :])
            x = acts.tile([P, CD, T], bf16, tag="x")

            def mk_copy_evict(dst, bias):
                def ev(m, ps_ap):
                    nc.scalar.activation(out=dst[:, m, :], in_=ps_ap, func=AF.Identity,
                                         bias=bias[:, m:m + 1], scale=1.0)
                return ev

            gemm(fc_w[:, :], e_n, CD, CD, mk_copy_evict(x, fcb))

            dump("x0", x[:, :, :])

            # ---- transformer layers ----
            for l in range(L):
                l1g = ldvec(ln1_g[l]); l1b = ldvec(ln1_b[l])
                h = acts.tile([P, CD, T], bf16, tag="h")
                layer_norm(x, l1g, l1b, h)

                if l == 0:
                    dump("h0", h[:, :, :])
                qb = ldvec(wq_b[l]); kb = ldvec(wk_b[l])
                q = acts.tile([P, CD, T], bf16, tag="q")
                k = acts.tile([P, CD, T], bf16, tag="k")
                gemm(wq_w[l], h, CD, CD, mk_copy_evict(q, qb))
                gemm(wk_w[l], h, CD, CD, mk_copy_evict(k, kb))

                # V in token-major layout: v_tok[t, dv]
                v_tok = acts.tile([P, TQ, H * DVh], bf16, tag="vtok")
                v_ps = [psA.tile([P, 2, T], f32, tag=f"gps{i}", name=f"gps{i}") for i in range(4)]
                for c in range(CD):
                    wsl = wslp.tile([P, H * DVh], bf16, tag="wslab")
                    nc.sync.dma_start(out=wsl, in_=wv_w[l, c * P:(c + 1) * P, :])
                    for tt in range(TQ):
                        for nb_ in range(2):
                            nc.tensor.matmul(v_ps[tt * 2 + nb_][:, :, :],
                                             lhsT=h[:, c, tt * P:(tt + 1) * P],
                                             rhs=wsl[:, nb_ * 512:(nb_ + 1) * 512],
                                             start=(c == 0), stop=(c == CD - 1))
                for tt in range(TQ):
                    for nb_ in range(2):
                        nc.scalar.activation(out=v_tok[:, tt, nb_ * 512:(nb_ + 1) * 512],
                                             in_=v_ps[tt * 2 + nb_][:, :, :], func=AF.Copy,
                                             bias=0.0, scale=1.0)

                if l == 0:
                    dump("q0", q[:, :, :])
                    dump("k0", k[:, :, :])
                    dump("vtok0", v_tok[:, :, :])

                # ---- attention, two heads (one dm-chunk) at a time ----
                vb = ldvec(wv_b[l])
                ctx = acts.tile([P, CD, T], bf16, tag="ctx")
                for hp in range(CD):
                    sc = psB.tile([P, 4, T], f32, tag="sc")
                    for hl in range(2):
                        qs = q[hl * 64:hl * 64 + 64, hp, :]
                        ks = k[hl * 64:hl * 64 + 64, hp, :]
                        for qt in range(TQ):
                            nc.tensor.matmul(sc[:, hl * 2 + qt, :], lhsT=qs[:, qt * P:(qt + 1) * P],
                                             rhs=ks[:, :], start=True, stop=True)
                    zm = sm.tile([P, 4, T], f32, tag="zm")
                    nc.vector.tensor_tensor(out=zm[:, :, :], in0=sc[:, :, :], in1=mask_sb[:, :, :], op=OP.add)
                    e = sm.tile([P, 4, T], bf16, tag="e")
                    nc.scalar.activation(out=e[:, :, :], in_=zm[:, :, :], func=AF.Exp, bias=0.0, scale=float(SCALE))
                    ssum = sm.tile([P, 4], f32, tag="ssum")
                    nc.vector.reduce_sum(out=ssum[:, :], in_=e[:, :, :], axis=AX.X)
                    nc.vector.reciprocal(out=ssum[:, :], in_=ssum[:, :])
                    at = sm.tile([P, 4, T], bf16, tag="at")
                    for j in range(4):
                        nc.vector.tensor_scalar_mul(at[:, j, :], e[:, j, :], ssum[:, j:j + 1])
                    # transpose attn -> [k, q] per head, then ctx
                    atT = sm.tile([P, 2, 2, T], bf16, tag="atT")
                    for hl in range(2):
                        for kt in range(TQ):
                            tp = psB.tile([P, 2, P], bf16, tag="tp")
                            for qt in range(TQ):
                                nc.tensor.transpose(tp[:, qt, :], at[:, hl * 2 + qt, kt * P:(kt + 1) * P],
                                                    ident[:, :])
                            nc.scalar.activation(out=atT[:, hl, kt, :], in_=tp[:, :, :], func=AF.Copy,
                                                 bias=0.0, scale=1.0)
                    if l == 0 and hp == 0:
                        dump("at0", at[:, :, :])
                    cp = psB.tile([P, T], f32, tag="cps")
                    for hl in range(2):
                        dv0 = (2 * hp + hl) * DVh
                        for kt in range(TQ):
                            nc.tensor.matmul(cp[hl * 64:hl * 64 + 64, :],
                                             lhsT=v_tok[:, kt, dv0:dv0 + DVh],
                                             rhs=atT[:, hl, kt, :], start=(kt == 0), stop=(kt == 1))
                    nc.scalar.activation(out=ctx[:, hp, :], in_=cp[:, :], func=AF.Identity,
                                         bias=vb[:, hp:hp + 1], scale=1.0)

                # ---- attention out + residual(h) ----
                aob = ldvec(ao_b[l])
                x2 = acts.tile([P, CD, T], bf16, tag="x2")

                def ao_evict(m, ps_ap):
                    t = sm.tile([P, T], f32, tag="aot")
                    nc.vector.tensor_tensor(out=t[:, :], in0=ps_ap, in1=h[:, m, :], op=OP.add)
                    nc.vector.tensor_scalar_add(x2[:, m, :], t[:, :], aob[:, m:m + 1])

                gemm(ao_w[l], ctx, CD, CD, ao_evict)

                if l == 0:
                    dump("ctx0", ctx[:, :, :])
                    dump("x2_0", x2[:, :, :])
                l2g = ldvec(ln2_g[l]); l2b = ldvec(ln2_b[l])
                h2 = acts.tile([P, CD, T], bf16, tag="h2")
                layer_norm(x2, l2g, l2b, h2)

                # ---- FFN ----
                if l == 0:
                    dump("h2_0", h2[:, :, :])
                f1b = ldvec(f1_b[l])
                h1 = acts.tile([P, CF, T], bf16, tag="h1")
                for blk in range(4):
                    def f1_evict(m, ps_ap, blk=blk):
                        mg = blk * 8 + m
                        nc.scalar.activation(out=h1[:, mg, :], in_=ps_ap, func=AF.Gelu,
                                             bias=f1b[:, mg:mg + 1], scale=1.0)
                    gemm(f1_w[l][:, blk * 1024:(blk + 1) * 1024], h2, CD, CD, f1_evict)

                f2b = ldvec(f2_b[l])
                xn = acts.tile([P, CD, T], bf16, tag="x")
                gemm(f2_w[l], h1, CF, CD, mk_copy_evict(xn, f2b))
                x = xn
                if l == 0:
                    dump("x1", x[:, :, :])

            # ---- final LN, gather, head ----
            lfg = ldvec(lnf_g[:, :]); lfb = ldvec(lnf_b[:, :])
            xf = acts.tile([P, CD, T], bf16, tag="xf")
            layer_norm(x, lfg, lfb, xf)

            dump("xf", xf[:, :, :])
            x_tok = acts.tile([P, TQ, DM], bf16, tag="xtok")
            for tt in range(TQ):
                for cg in range(2):
                    tp2 = psB.tile([P, 4, P], bf16, tag="sc")
                    for ci in range(4):
                        c = cg * 4 + ci
                        nc.tensor.transpose(tp2[:, ci, :], xf[:, c, tt * P:(tt + 1) * P], ident[:, :])
                    nc.scalar.activation(out=x_tok[:, tt, cg * 512:(cg + 1) * 512], in_=tp2[:, :, :],
                                         func=AF.Copy, bias=0.0, scale=1.0)

            dump("xtok", x_tok[:, :, :])
            poh = cst.tile([P, TQ, NM], bf16, tag="poh")
            nc.sync.dma_start(out=poh, in_=pos_oh[:, :].rearrange("(tt p) m -> p tt m", p=P))
            sel_ps = psB.tile([P, CD, NM], f32, tag="sc")
            for c in range(CD):
                for tt in range(TQ):
                    nc.tensor.matmul(sel_ps[:, c, :], lhsT=x_tok[:, tt, c * P:(c + 1) * P],
                                     rhs=poh[:, tt, :], start=(tt == 0), stop=(tt == 1))
            sel = sm.tile([P, CD, NM], bf16, tag="sel")
            nc.scalar.activation(out=sel[:, :, :], in_=sel_ps[:, :, :], func=AF.Copy, bias=0.0, scale=1.0)

            dump("sel", sel[:, :, :])
            linb = ldvec(lin_b[:, :])
            lin_ps = psA.tile([P, CD, NM], f32, tag="gps0")
            for c in range(CD):
                wsl = wslp.tile([P, DM], bf16, tag="wslab")
                nc.sync.dma_start(out=wsl, in_=lin_w[c * P:(c + 1) * P, :])
                for m in range(CD):
                    nc.tensor.matmul(lin_ps[:, m, :], lhsT=wsl[:, m * P:(m + 1) * P],
                                     rhs=sel[:, c, :], start=(c == 0 and m == 0),
                                     stop=(c == CD - 1 and m == CD - 1))
            hg = sm.tile([P, CD, NM], bf16, tag="hg")
            for m in range(CD):
                nc.scalar.activation(out=hg[:, m, :], in_=lin_ps[:, m, :], func=AF.Gelu,
                                     bias=linb[:, m:m + 1], scale=1.0)

            dump("hg", hg[:, :, :])
            hw_sb = cst.tile([P, CD, V], bf16, tag="hw")
            nc.sync.dma_start(out=hw_sb, in_=head_w[:, :].rearrange("(c p) v -> p c v", p=P))
            lg_ps = psB.tile([V, NM], f32, tag="tp")
            for c in range(CD):
                nc.tensor.matmul(lg_ps[:, :], lhsT=hw_sb[:, c, :], rhs=hg[:, c, :],
                                 start=(c == 0), stop=(c == CD - 1))
            lgT = sm.tile([V, NM], f32, tag="lgT")
            nc.vector.tensor_copy(lgT[:, :], lg_ps[:, :])
            nc.sync.dma_start(out=logits_out[:, :], in_=lgT[:, :])

    nc.finalize()
    return nc


def _col_layout(vec, nch):
    """[D] -> [P, D//P] so column m is the per-partition bias of chunk m."""
    return np.ascontiguousarray(np.asarray(vec, np.float32).reshape(nch, P).T)


def prepare_inputs(inputs):
    f = {k: np.asarray(v) for k, v in inputs.items()}
    wmap = {
        "te_w": f["tok_embed"].astype(BF),
        "pos_eT": np.ascontiguousarray(f["pos_embed"][0].T).astype(BF),
        "fc_w": f["fc_w"].astype(BF),
        "wq_w": f["wq_w"].astype(BF),
        "wk_w": f["wk_w"].astype(BF),
        "wv_w": f["wv_w"].astype(BF),
        "ao_w": f["ao_w"].astype(BF),
        "f1_w": f["f1_w"].astype(BF),
        "f2_w": f["f2_w"].astype(BF),
        "lin_w": f["lin_w"].astype(BF),
        "head_w": f["head_w"].astype(BF),
        "norm_g": _col_layout(f["norm_g"], CD),
        "norm_b": _col_layout(f["norm_b"], CD),
        "fc_b": _col_layout(f["fc_b"], CD),
        "lnf_g": _col_layout(f["lnf_g"], CD),
        "lnf_b": _col_layout(f["lnf_b"], CD),
        "lin_b": _col_layout(f["lin_b"], CD),
        "ln1_g": np.stack([_col_layout(f["ln1_g"][l], CD) for l in range(L)]),
        "ln1_b": np.stack([_col_layout(f["ln1_b"][l], CD) for l in range(L)]),
        "ln2_g": np.stack([_col_layout(f["ln2_g"][l], CD) for l in range(L)]),
        "ln2_b": np.stack([_col_layout(f["ln2_b"][l], CD) for l in range(L)]),
        "wq_b": np.stack([_col_layout(f["wq_b"][l], CD) for l in range(L)]),
        "wk_b": np.stack([_col_layout(f["wk_b"][l], CD) for l in range(L)]),
        "wv_b": np.stack([_col_layout(f["wv_b"][l], CD) for l in range(L)]),
        "ao_b": np.stack([_col_layout(f["ao_b"][l], CD) for l in range(L)]),
        "f1_b": np.stack([_col_layout(f["f1_b"][l], CF) for l in range(L)]),
        "f2_b": np.stack([_col_layout(f["f2_b"][l], CD) for l in range(L)]),
    }
    seqs = np.asarray(f["masked_seqs"]).astype(np.int64)
    poss = np.asarray(f["masked_pos"]).astype(np.int64)
    in_maps = []
    for b in range(B):
        seq_oh = (seqs[b][None, :] == np.arange(V)[:, None]).astype(BF)
        pos_oh = (poss[b][None, :] == np.arange(T)[:, None]).astype(BF)
        maskrow = np.where(seqs[b] == 0, np.float32(MASKVAL), np.float32(0.0))
        mask4 = np.broadcast_to(maskrow[None, None, :], (P, 4, T)).astype(np.float32)
        in_maps.append({"seq_oh": seq_oh, "pos_oh": np.ascontiguousarray(pos_oh),
                        "mask4": np.ascontiguousarray(mask4), **wmap})
    return in_maps


_GRAPH_CACHE = {}


def run(inputs, trace=False):
    in_maps = prepare_inputs(inputs)
    if "nc" not in _GRAPH_CACHE:
        _GRAPH_CACHE["nc"] = build_graph()
    nc = _GRAPH_CACHE["nc"]
    res = run_bass_kernel_spmd(nc, in_maps, core_ids=list(range(B)), trace=trace)
    logits = np.stack([np.ascontiguousarray(r["logitsT"].T) for r in res.results])
    # loss on host (exact replica of the reference math, in fp32)
    lg = logits.astype(np.float32)
    mx = lg.max(-1, keepdims=True)
    logp = lg - mx - np.log(np.exp(lg - mx).sum(-1, keepdims=True))
    tok = np.asarray(inputs["masked_tokens"]).astype(np.int64)
    picked = np.take_along_axis(logp, tok[:, :, None], axis=-1)
    loss = np.float32(-picked.mean())
    return (logits, loss), res


def kernel(**inputs):
    (logits, loss), _ = run(inputs, trace=False)
    return logits, loss


# revision 8
# speedup vs baseline: 1.2399x; 1.2399x over previous
"""BERT-style MLM forward on 8 TRN2 NeuronCores.

Strategy: pure data-parallel over batch (B=8 -> 1 sequence per core, no
collectives). Feature-major activations [dm, t] so every GEMM uses the
natural weight layout as the stationary operand. bf16 matmul operands with
fp32 PSUM accumulation; LN/softmax statistics in fp32. Embedding lookup and
masked-position gather are done as one-hot matmuls (one-hots built on host).
The final loss scalar is derived on host from the device-computed logits
(identical math to the reference, ~10K FLOPs).
"""
import sys

try:
    import concourse.bass as bass  # noqa: F401
except ImportError:
    sys.path.insert(0, "/opt/trn_rl_repo")

import ml_dtypes
import numpy as np

import concourse.bass as bass
import concourse.mybir as mybir
from concourse import bacc, tile
from concourse.bass_utils import run_bass_kernel_spmd
from concourse.masks import make_identity

B, T, NM = 8, 256, 40
V, NE, DM, DK, DVh, H, DFF, L = 30, 1024, 1024, 64, 64, 16, 4096, 8
EPS = 1e-5
P = 128
CD = DM // P   # 8 chunks of the model dim
CF = DFF // P  # 32 chunks of the ffn dim
TQ = T // P    # 2 token tiles
SCALE = 1.0 / np.sqrt(DK).astype(np.float32)  # 0.125
MASKVAL = float(-1e9 / SCALE)

bf16 = mybir.dt.bfloat16
f32 = mybir.dt.float32
BF = ml_dtypes.bfloat16
AF = mybir.ActivationFunctionType
OP = mybir.AluOpType
AX = mybir.AxisListType


def build_graph(debug=False):
    nc = bacc.Bacc()
    dp = {}
    dbg_outs = {}

    def param(name, shape, dt=bf16):
        dp[name] = nc.declare_dram_parameter(name, list(shape), dt, isOutput=False)
        return dp[name]

    # per-core tensors
    seq_oh = param("seq_oh", (V, T))            # one-hot of masked_seqs[b]
    pos_oh = param("pos_oh", (T, NM))           # one-hot of masked_pos[b]
    mask4 = param("mask4", (P, 4, T))           # multiplicative 0/1 pad mask, replicated
    # weights (shared across cores)
    te_w = param("te_w", (V, NE))
    pos_eT = param("pos_eT", (NE, T))           # pos_embed[0].T (feature-major)
    fc_w = param("fc_w", (NE, DM))
    wq_w = param("wq_w", (L, DM, H * DK))
    wk_w = param("wk_w", (L, DM, H * DK))
    wv_w = param("wv_w", (L, DM, H * DVh))
    ao_w = param("ao_w", (L, H * DVh, DM))
    f1_w = param("f1_w", (L, DM, DFF))
    f2_w = param("f2_w", (L, DFF, DM))
    lin_w = param("lin_w", (DM, DM))
    head_w = param("head_w", (DM, V))
    # fp32 per-partition-layout vectors [P, nchunks]
    norm_g = param("norm_g", (P, CD), f32)
    norm_b = param("norm_b", (P, CD), f32)
    fc_b = param("fc_b", (P, CD), f32)
    ln1_g = param("ln1_g", (L, P, CD), f32)
    ln1_b = param("ln1_b", (L, P, CD), f32)
    ln2_g = param("ln2_g", (L, P, CD), f32)
    ln2_b = param("ln2_b", (L, P, CD), f32)
    wq_b = param("wq_b", (L, P, CD), f32)
    wk_b = param("wk_b", (L, P, CD), f32)
    wv_b = param("wv_b", (L, P, CD), f32)
    ao_b = param("ao_b", (L, P, CD), f32)
    f1_b = param("f1_b", (L, P, CF), f32)
    f2_b = param("f2_b", (L, P, CD), f32)
    lnf_g = param("lnf_g", (P, CD), f32)
    lnf_b = param("lnf_b", (P, CD), f32)
    lin_b = param("lin_b", (P, CD), f32)
    logits_out = nc.declare_dram_parameter("logitsT", [V, NM], f32, isOutput=True)

    def mkdump(name, shape):
        if debug:
            dbg_outs[name] = nc.declare_dram_parameter("dbg_" + name, list(shape), bf16, isOutput=True)

    for nm in ("emb", "e_n", "x0", "h0", "q0", "k0", "ctx0", "x2_0", "h2_0", "x1", "xf"):
        if debug:
            dbg_outs[nm] = nc.declare_dram_parameter("dbg_" + nm, [P, CD, T], bf16, isOutput=True)
    if debug:
        dbg_outs["vtok0"] = nc.declare_dram_parameter("dbg_vtok0", [P, TQ, H * DVh], bf16, isOutput=True)
        dbg_outs["xtok"] = nc.declare_dram_parameter("dbg_xtok", [P, TQ, DM], bf16, isOutput=True)
        dbg_outs["sel"] = nc.declare_dram_parameter("dbg_sel", [P, CD, NM], bf16, isOutput=True)
        dbg_outs["hg"] = nc.declare_dram_parameter("dbg_hg", [P, CD, NM], bf16, isOutput=True)
        dbg_outs["at0"] = nc.declare_dram_parameter("dbg_at0", [P, 4, T], bf16, isOutput=True)

    with tile.TileContext(nc) as tc:
        with tc.tile_pool(name="cst", bufs=1) as cst, \
             tc.tile_pool(name="acts", bufs=2) as acts, \
             tc.tile_pool(name="wsl", bufs=36) as wslp, \
             tc.tile_pool(name="sm", bufs=2) as sm, \
             tc.tile_pool(name="psA", bufs=4, space="PSUM") as psA, \
             tc.tile_pool(name="psB", bufs=1, space="PSUM") as psB:

            # ---- constants ----
            ident = cst.tile([P, P], bf16, tag="ident")
            make_identity(nc, ident)
            onesk = cst.tile([P, 1], bf16, tag="onesk")
            nc.vector.memset(onesk, 1.0)
            ones1 = cst.tile([1, P], bf16, tag="ones1")
            nc.vector.memset(ones1, 1.0)
            ones1f = cst.tile([1, P], f32, tag="ones1f")
            nc.vector.memset(ones1f, 1.0)
            eps_t = cst.tile([1, 1], f32, tag="eps")
            nc.vector.memset(eps_t, EPS)
            mask_sb = cst.tile([P, 4, T], bf16, tag="mask")
            nc.sync.dma_start(out=mask_sb, in_=mask4[:, :, :])

            def dump(name, tile_ap):
                if debug and name in dbg_outs:
                    nc.sync.dma_start(out=dbg_outs[name][:], in_=tile_ap)

            def ldvec(ap2d):
                t = sm.tile([P, ap2d.shape[-1]], f32, tag="vecs")
                nc.sync.dma_start(out=t, in_=ap2d)
                return t

            # ---- layernorm over the feature (partition-chunk) axis ----
            def layer_norm(x, g, b, out):
                sq = sm.tile([P, CD, T], bf16, tag="lnsq", bufs=1)
                nc.vector.tensor_tensor(out=sq[:, :, :], in0=x[:, :, :], in1=x[:, :, :], op=OP.mult)
                st = psB.tile([1, 2, T], f32, tag="tp")
                for c in range(CD):
                    nc.tensor.matmul(st[:, 0, :], lhsT=onesk[:, :], rhs=x[:, c, :],
                                     start=(c == 0), stop=(c == CD - 1))
                for c in range(CD):
                    nc.tensor.matmul(st[:, 1, :], lhsT=onesk[:, :], rhs=sq[:, c, :],
                                     start=False, stop=(c == CD - 1))
                mv = sm.tile([1, 2, T], f32, tag="ln_mv")
                nc.vector.tensor_scalar_mul(mv[:, :, :], st[:, :, :], 1.0 / DM)
                m2 = sm.tile([1, T], f32, tag="ln_m2")
                nc.vector.tensor_tensor(out=m2[:, :], in0=mv[:, 0, :], in1=mv[:, 0, :], op=OP.mult)
                nc.vector.tensor_tensor(out=m2[:, :], in0=mv[:, 1, :], in1=m2[:, :], op=OP.subtract)
                std = sm.tile([1, T], f32, tag="ln_std")
                nc.scalar.activation(out=std[:, :], in_=m2[:, :], func=AF.Sqrt,
                                     bias=eps_t[:1, :], scale=1.0)
                nc.vector.reciprocal(out=std[:, :], in_=std[:, :])
                bc = psA.tile([P, 2, T], f32, tag="gps", name="bc")
                nc.tensor.matmul(bc[:, 0, :], lhsT=ones1f[:, :], rhs=mv[:, 0, :], start=True, stop=True)
                nc.tensor.matmul(bc[:, 1, :], lhsT=ones1f[:, :], rhs=std[:, :], start=True, stop=True)
                xc = sm.tile([P, CD, T], bf16, tag="ln_xc")
                for c in range(CD):
                    nc.vector.tensor_tensor(out=xc[:, c, :], in0=x[:, c, :], in1=bc[:, 0, :], op=OP.subtract)
                    nc.vector.tensor_tensor(out=xc[:, c, :], in0=xc[:, c, :], in1=bc[:, 1, :], op=OP.mult)
                    nc.vector.tensor_scalar(out=out[:, c, :], in0=xc[:, c, :],
                                            scalar1=g[:, c:c + 1], scalar2=b[:, c:c + 1],
                                            op0=OP.mult, op1=OP.add)

            # ---- generic feature-major GEMM block: out chunks = W[cin*P, mout*P].T @ act ----
            # m-outer with all k-slabs resident: each m's psum completes early so its
            # eviction overlaps the next m's accumulation (keeps PE dense).
            def gemm(w_dram, act_in, cin, mout, evict):
                slabs = []
                for c in range(cin):
                    wsl = wslp.tile([P, mout * P], bf16, tag="wslab", name="wsl")
                    nc.sync.dma_start(out=wsl, in_=w_dram[c * P:(c + 1) * P, :])
                    slabs.append(wsl)
                for m in range(mout):
                    ps = psA.tile([P, T], f32, tag="gps", name="gps")
                    for c in range(cin):
                        nc.tensor.matmul(ps[:, :], lhsT=slabs[c][:, m * P:(m + 1) * P],
                                         rhs=act_in[:, c, :], start=(c == 0), stop=(c == cin - 1))
                    evict(m, ps[:, :])

            # ---- embedding ----
            seq_sb = cst.tile([V, T], bf16, tag="seq")
            nc.sync.dma_start(out=seq_sb, in_=seq_oh[:, :])
            te_sb = cst.tile([V, NE], bf16, tag="te")
            nc.sync.dma_start(out=te_sb, in_=te_w[:, :])
            pos_sb = cst.tile([P, CD, T], bf16, tag="pos")
            nc.sync.dma_start(out=pos_sb, in_=pos_eT[:, :].rearrange("(c p) t -> p c t", p=P))

            emb = acts.tile([P, CD, T], bf16, tag="x2")
            for c in range(CD):
                eps_ps = psA.tile([P, T], f32, tag="gps", name="gps")
                nc.tensor.matmul(eps_ps[:, :], lhsT=te_sb[:, c * P:(c + 1) * P],
                                 rhs=seq_sb[:, :], start=True, stop=True)
                nc.vector.tensor_tensor(out=emb[:, c, :], in0=eps_ps[:, :],
                                        in1=pos_sb[:, c, :], op=OP.add)

            dump("emb", emb[:, :, :])
            ng = ldvec(norm_g[:, :]); nb = ldvec(norm_b[:, :])
            e_n = acts.tile([P, CD, T], bf16, tag="h")
            layer_norm(emb, ng, nb, e_n)

            dump("e_n", e_n[:, :, :])
            fcb = ldvec(fc_b[:, :])
            x = acts.tile([P, CD, T], bf16, tag="x")

            def mk_copy_evict(dst, bias):
                def ev(m, ps_ap):
                    nc.vector.tensor_scalar_add(dst[:, m, :], ps_ap, bias[:, m:m + 1])
                return ev

            gemm(fc_w[:, :], e_n, CD, CD, mk_copy_evict(x, fcb))

            dump("x0", x[:, :, :])

            # ---- transformer layers ----
            for l in range(L):
                l1g = ldvec(ln1_g[l]); l1b = ldvec(ln1_b[l])
                h = acts.tile([P, CD, T], bf16, tag="h")
                layer_norm(x, l1g, l1b, h)

                if l == 0:
                    dump("h0", h[:, :, :])
                qb = ldvec(wq_b[l]); kb = ldvec(wk_b[l])
                q = acts.tile([P, CD, T], bf16, tag="q")
                k = acts.tile([P, CD, T], bf16, tag="k")
                gemm(wq_w[l], h, CD, CD, mk_copy_evict(q, qb))
                gemm(wk_w[l], h, CD, CD, mk_copy_evict(k, kb))

                # V in token-major layout: v_tok[t, dv]
                v_tok = acts.tile([P, TQ, H * DVh], bf16, tag="vtok")
                vslabs = []
                for c in range(CD):
                    wsl = wslp.tile([P, H * DVh], bf16, tag="wslab", name="wsl")
                    nc.sync.dma_start(out=wsl, in_=wv_w[l, c * P:(c + 1) * P, :])
                    vslabs.append(wsl)
                for tt in range(TQ):
                    for nb_ in range(2):
                        v_ps = psA.tile([P, 2, T], f32, tag="gps", name="vps")
                        for c in range(CD):
                            nc.tensor.matmul(v_ps[:, :, :],
                                             lhsT=h[:, c, tt * P:(tt + 1) * P],
                                             rhs=vslabs[c][:, nb_ * 512:(nb_ + 1) * 512],
                                             start=(c == 0), stop=(c == CD - 1))
                        nc.scalar.activation(out=v_tok[:, tt, nb_ * 512:(nb_ + 1) * 512],
                                             in_=v_ps[:, :, :], func=AF.Copy,
                                             bias=0.0, scale=1.0)

                if l == 0:
                    dump("q0", q[:, :, :])
                    dump("k0", k[:, :, :])
                    dump("vtok0", v_tok[:, :, :])

                # ---- attention, two heads (one dm-chunk) at a time ----
                vb = ldvec(wv_b[l])
                ctx = acts.tile([P, CD, T], bf16, tag="ctx")
                for hp in range(CD):
                    sc = psB.tile([P, 4, T], f32, tag="sc")
                    for hl in range(2):
                        qs = q[hl * 64:hl * 64 + 64, hp, :]
                        ks = k[hl * 64:hl * 64 + 64, hp, :]
                        for qt in range(TQ):
                            nc.tensor.matmul(sc[:, hl * 2 + qt, :], lhsT=qs[:, qt * P:(qt + 1) * P],
                                             rhs=ks[:, :], start=True, stop=True)
                    e = sm.tile([P, 4, T], bf16, tag="e")
                    nc.scalar.activation(out=e[:, :, :], in_=sc[:, :, :], func=AF.Exp, bias=0.0, scale=float(SCALE))
                    nc.vector.tensor_tensor(out=e[:, :, :], in0=e[:, :, :], in1=mask_sb[:, :, :], op=OP.mult)
                    ssum = sm.tile([P, 4], f32, tag="ssum")
                    nc.vector.reduce_sum(out=ssum[:, :], in_=e[:, :, :], axis=AX.X)
                    nc.vector.reciprocal(out=ssum[:, :], in_=ssum[:, :])
                    at = sm.tile([P, 4, T], bf16, tag="at")
                    for j in range(4):
                        nc.vector.tensor_scalar_mul(at[:, j, :], e[:, j, :], ssum[:, j:j + 1])
                    # transpose attn -> [k, q] per head, then ctx
                    atT = sm.tile([P, 2, 2, T], bf16, tag="atT")
                    for hl in range(2):
                        for kt in range(TQ):
                            tp = psB.tile([P, 2, P], bf16, tag="tp")
                            for qt in range(TQ):
                                nc.tensor.transpose(tp[:, qt, :], at[:, hl * 2 + qt, kt * P:(kt + 1) * P],
                                                    ident[:, :])
                            nc.scalar.activation(out=atT[:, hl, kt, :], in_=tp[:, :, :], func=AF.Copy,
                                                 bias=0.0, scale=1.0)
                    if l == 0 and hp == 0:
                        dump("at0", at[:, :, :])
                    cp = psB.tile([P, T], f32, tag="cps")
                    for hl in range(2):
                        dv0 = (2 * hp + hl) * DVh
                        for kt in range(TQ):
                            nc.tensor.matmul(cp[hl * 64:hl * 64 + 64, :],
                                             lhsT=v_tok[:, kt, dv0:dv0 + DVh],
                                             rhs=atT[:, hl, kt, :], start=(kt == 0), stop=(kt == 1))
                    nc.vector.tensor_scalar_add(ctx[:, hp, :], cp[:, :], vb[:, hp:hp + 1])

                # ---- attention out + residual(h) ----
                aob = ldvec(ao_b[l])
                x2 = acts.tile([P, CD, T], bf16, tag="x2")

                def ao_evict(m, ps_ap):
                    t = sm.tile([P, T], f32, tag="aot")
                    nc.vector.tensor_tensor(out=t[:, :], in0=ps_ap, in1=h[:, m, :], op=OP.add)
                    nc.vector.tensor_scalar_add(x2[:, m, :], t[:, :], aob[:, m:m + 1])

                gemm(ao_w[l], ctx, CD, CD, ao_evict)

                if l == 0:
                    dump("ctx0", ctx[:, :, :])
                    dump("x2_0", x2[:, :, :])
                l2g = ldvec(ln2_g[l]); l2b = ldvec(ln2_b[l])
                h2 = acts.tile([P, CD, T], bf16, tag="h2")
                layer_norm(x2, l2g, l2b, h2)

                # ---- FFN ----
                if l == 0:
                    dump("h2_0", h2[:, :, :])
                f1b = ldvec(f1_b[l])
                h1 = acts.tile([P, CF, T], bf16, tag="h1", bufs=1)
                for blk in range(4):
                    def f1_evict(m, ps_ap, blk=blk):
                        mg = blk * 8 + m
                        nc.scalar.activation(out=h1[:, mg, :], in_=ps_ap, func=AF.Gelu,
                                             bias=f1b[:, mg:mg + 1], scale=1.0)
                    gemm(f1_w[l][:, blk * 1024:(blk + 1) * 1024], h2, CD, CD, f1_evict)

                f2b = ldvec(f2_b[l])
                xn = acts.tile([P, CD, T], bf16, tag="x")
                gemm(f2_w[l], h1, CF, CD, mk_copy_evict(xn, f2b))
                x = xn
                if l == 0:
                    dump("x1", x[:, :, :])

            # ---- final LN, gather, head ----
            lfg = ldvec(lnf_g[:, :]); lfb = ldvec(lnf_b[:, :])
            xf = acts.tile([P, CD, T], bf16, tag="x2")
            layer_norm(x, lfg, lfb, xf)

            dump("xf", xf[:, :, :])
            x_tok = acts.tile([P, TQ, DM], bf16, tag="vtok")
            for tt in range(TQ):
                for cg in range(2):
                    tp2 = psB.tile([P, 4, P], bf16, tag="sc")
                    for ci in range(4):
                        c = cg * 4 + ci
                        nc.tensor.transpose(tp2[:, ci, :], xf[:, c, tt * P:(tt + 1) * P], ident[:, :])
                    nc.scalar.activation(out=x_tok[:, tt, cg * 512:(cg + 1) * 512], in_=tp2[:, :, :],
                                         func=AF.Copy, bias=0.0, scale=1.0)

            dump("xtok", x_tok[:, :, :])
            poh = cst.tile([P, TQ, NM], bf16, tag="poh")
            nc.sync.dma_start(out=poh, in_=pos_oh[:, :].rearrange("(tt p) m -> p tt m", p=P))
            sel_ps = psB.tile([P, CD, NM], f32, tag="sc")
            for c in range(CD):
                for tt in range(TQ):
                    nc.tensor.matmul(sel_ps[:, c, :], lhsT=x_tok[:, tt, c * P:(c + 1) * P],
                                     rhs=poh[:, tt, :], start=(tt == 0), stop=(tt == 1))
            sel = sm.tile([P, CD, NM], bf16, tag="sel")
            nc.scalar.activation(out=sel[:, :, :], in_=sel_ps[:, :, :], func=AF.Copy, bias=0.0, scale=1.0)

            dump("sel", sel[:, :, :])
            linb = ldvec(lin_b[:, :])
            lslabs = []
            for c in range(CD):
                wsl = wslp.tile([P, DM], bf16, tag="wslab", name="wsl")
                nc.sync.dma_start(out=wsl, in_=lin_w[c * P:(c + 1) * P, :])
                lslabs.append(wsl)
            hg = sm.tile([P, CD, NM], bf16, tag="hg")
            for m in range(CD):
                lin_ps = psA.tile([P, NM], f32, tag="gps", name="lps")
                for c in range(CD):
                    nc.tensor.matmul(lin_ps[:, :], lhsT=lslabs[c][:, m * P:(m + 1) * P],
                                     rhs=sel[:, c, :], start=(c == 0), stop=(c == CD - 1))
                nc.scalar.activation(out=hg[:, m, :], in_=lin_ps[:, :], func=AF.Gelu,
                                     bias=linb[:, m:m + 1], scale=1.0)

            dump("hg", hg[:, :, :])
            hw_sb = cst.tile([P, CD, V], bf16, tag="hw")
            nc.sync.dma_start(out=hw_sb, in_=head_w[:, :].rearrange("(c p) v -> p c v", p=P))
            lg_ps = psB.tile([V, NM], f32, tag="tp")
            for c in range(CD):
                nc.tensor.matmul(lg_ps[:, :], lhsT=hw_sb[:, c, :], rhs=hg[:, c, :],
                                 start=(c == 0), stop=(c == CD - 1))
            lgT = sm.tile([V, NM], f32, tag="lgT")
            nc.vector.tensor_copy(lgT[:, :], lg_ps[:, :])
            nc.sync.dma_start(out=logits_out[:, :], in_=lgT[:, :])

    nc.finalize()
    return nc


def _col_layout(vec, nch):
    """[D] -> [P, D//P] so column m is the per-partition bias of chunk m."""
    return np.ascontiguousarray(np.asarray(vec, np.float32).reshape(nch, P).T)


def prepare_inputs(inputs):
    f = {k: np.asarray(v) for k, v in inputs.items()}
    wmap = {
        "te_w": f["tok_embed"].astype(BF),
        "pos_eT": np.ascontiguousarray(f["pos_embed"][0].T).astype(BF),
        "fc_w": f["fc_w"].astype(BF),
        "wq_w": f["wq_w"].astype(BF),
        "wk_w": f["wk_w"].astype(BF),
        "wv_w": f["wv_w"].astype(BF),
        "ao_w": f["ao_w"].astype(BF),
        "f1_w": f["f1_w"].astype(BF),
        "f2_w": f["f2_w"].astype(BF),
        "lin_w": f["lin_w"].astype(BF),
        "head_w": f["head_w"].astype(BF),
        "norm_g": _col_layout(f["norm_g"], CD),
        "norm_b": _col_layout(f["norm_b"], CD),
        "fc_b": _col_layout(f["fc_b"], CD),
        "lnf_g": _col_layout(f["lnf_g"], CD),
        "lnf_b": _col_layout(f["lnf_b"], CD),
        "lin_b": _col_layout(f["lin_b"], CD),
        "ln1_g": np.stack([_col_layout(f["ln1_g"][l], CD) for l in range(L)]),
        "ln1_b": np.stack([_col_layout(f["ln1_b"][l], CD) for l in range(L)]),
        "ln2_g": np.stack([_col_layout(f["ln2_g"][l], CD) for l in range(L)]),
        "ln2_b": np.stack([_col_layout(f["ln2_b"][l], CD) for l in range(L)]),
        "wq_b": np.stack([_col_layout(f["wq_b"][l], CD) for l in range(L)]),
        "wk_b": np.stack([_col_layout(f["wk_b"][l], CD) for l in range(L)]),
        "wv_b": np.stack([_col_layout(f["wv_b"][l], CD) for l in range(L)]),
        "ao_b": np.stack([_col_layout(f["ao_b"][l], CD) for l in range(L)]),
        "f1_b": np.stack([_col_layout(f["f1_b"][l], CF) for l in range(L)]),
        "f2_b": np.stack([_col_layout(f["f2_b"][l], CD) for l in range(L)]),
    }
    seqs = np.asarray(f["masked_seqs"]).astype(np.int64)
    poss = np.asarray(f["masked_pos"]).astype(np.int64)
    in_maps = []
    for b in range(B):
        seq_oh = (seqs[b][None, :] == np.arange(V)[:, None]).astype(BF)
        pos_oh = (poss[b][None, :] == np.arange(T)[:, None]).astype(BF)
        maskrow = np.where(seqs[b] == 0, np.float32(0.0), np.float32(1.0))
        mask4 = np.broadcast_to(maskrow[None, None, :], (P, 4, T)).astype(BF)
        in_maps.append({"seq_oh": seq_oh, "pos_oh": np.ascontiguousarray(pos_oh),
                        "mask4": np.ascontiguousarray(mask4), **wmap})
    return in_maps


_GRAPH_CACHE = {}


def run(inputs, trace=False):
    in_maps = prepare_inputs(inputs)
    if "nc" not in _GRAPH_CACHE:
        _GRAPH_CACHE["nc"] = build_graph()
    nc = _GRAPH_CACHE["nc"]
    res = run_bass_kernel_spmd(nc, in_maps, core_ids=list(range(B)), trace=trace)
    logits = np.stack([np.ascontiguousarray(r["logitsT"].T) for r in res.results])
    # loss on host (exact replica of the reference math, in fp32)
    lg = logits.astype(np.float32)
    mx = lg.max(-1, keepdims=True)
    logp = lg - mx - np.log(np.exp(lg - mx).sum(-1, keepdims=True))
    tok = np.asarray(inputs["masked_tokens"]).astype(np.int64)
    picked = np.take_along_axis(logp, tok[:, :, None], axis=-1)
    loss = np.float32(-picked.mean())
    return (logits, loss), res


def kernel(**inputs):
    (logits, loss), _ = run(inputs, trace=False)
    return logits, loss
